# revision 4
# baseline (speedup 1.0000x reference)
"""GCN (3-layer + readout) on 8 Trainium2 NeuronCores.

Strategy (dst-node sharding, 1D graph parallel):
  - Nodes are sharded across 8 cores (6250/core, padded to 6272 = 49 blocks
    of 128).  Each core aggregates messages for the edges whose dst lands in
    its shard.
  - Per layer: transform z = h @ W (PE), scale rows by dinv = deg^-1/2 so
    table rows are dinv[src]*z[src].  Layer 1's table is computed fully
    locally by every core (x is replicated; each core gets a node
    permutation with its own shard first so the self-term slice is uniform
    across the SPMD program).  Layers 2/3 AllGather the shard tables.
  - Edge gathers: one indirect DMA (standard InstDMACopy dynamic-offset
    SWDGE path, int32 row ids, one row per partition) per 128-edge chunk.
  - Scatter-add on the TensorEngine: per chunk a one-hot
    onehot[e, d] = (dst_local[e] == d) is built with one DVE
    tensor_scalar(is_equal) against an iota row; psum[128d, 64] +=
    onehot^T @ msg accumulates the dst-block aggregation.
  - Self loops (PyG GCNConv implicit): the own-shard table slice is kept in
    SBUF and added to the block aggregate before the dst-side dinv scale,
    yielding exactly dinv^2 * z.
  - Host-side preprocessing is strictly index/metadata work (edge bucketing,
    padding, degree counting, node permutation); all float math runs on
    device.
"""

import numpy as np

from concourse import bacc, bass, mybir, tile
from concourse.bass_utils import run_bass_kernel_spmd

# ---------------------------------------------------------------- constants
P = 8                      # cores
N = 50000                  # nodes
IN_DIM = 128
HID = 64
OUT_DIM = 10
BLK = 128

F32 = mybir.dt.float32
I32 = mybir.dt.int32


def _derived():
    shard = N // P
    nblk = (shard + BLK - 1) // BLK
    pads = nblk * BLK
    tbl = P * pads
    return shard, nblk, pads, tbl


def _refresh_dims():
    global SHARD, NBLK, PADS, TBL
    SHARD, NBLK, PADS, TBL = _derived()


SHARD, NBLK, PADS, TBL = _derived()


# ------------------------------------------------------------- host prep
def _preprocess(x, edge_index):
    """Bucket edges into per-(core, dst-block) 128-edge chunks."""
    x = np.asarray(x, np.float32)
    ei = np.asarray(edge_index, np.int64)
    src, dst = ei[0], ei[1]

    deg = np.bincount(dst, minlength=N).astype(np.float32) + 1.0

    owner = dst // SHARD
    blk = (dst - owner * SHARD) // BLK
    dstl = ((dst - owner * SHARD) % BLK).astype(np.float32)
    s_own = src // SHARD
    s_loc = src % SHARD

    # chunk slots
    gid = owner * NBLK + blk
    order = np.argsort(gid, kind="stable")
    gid_s = gid[order]
    counts = np.bincount(gid_s, minlength=P * NBLK)
    starts = np.concatenate([[0], np.cumsum(counts)[:-1]])
    pos = np.arange(gid_s.size) - starts[gid_s]

    C_BLK = int(np.ceil(counts.max() / BLK))
    T = NBLK * C_BLK

    own_s = gid_s // NBLK
    blk_s = gid_s % NBLK
    slot = blk_s * C_BLK * BLK + pos          # slot within the core's stream

    # AG-table row (rank-ordered layout, layers 2/3)
    row23 = (s_own * PADS + s_loc)[order]
    dstl_s = dstl[order]

    g23 = np.zeros((P, T * BLK), np.int32)
    dv = np.full((P, T * BLK), -1.0, np.float32)
    flat = own_s * (T * BLK) + slot
    g23.reshape(-1)[flat] = row23.astype(np.int32)
    dv.reshape(-1)[flat] = dstl_s

    # layer-1 table row: per-core permuted layout, own shard first
    g1 = np.zeros((P, T * BLK), np.int32)
    s_own_s = s_own[order]
    s_loc_s = s_loc[order]
    for k in range(P):
        sel = own_s == k
        so = s_own_s[sel]
        # position of shard `so` in core k's permuted order [k, 0,1,..(!k)..,7]
        rank = np.where(so == k, 0, 1 + so - (so > k))
        g1.reshape(-1)[flat[sel]] = (rank * PADS + s_loc_s[sel]).astype(np.int32)

    x_pad = np.zeros((P, PADS, IN_DIM), np.float32)
    deg_pad = np.ones((P, PADS), np.float32)
    for k in range(P):
        x_pad[k, :SHARD] = x[k * SHARD:(k + 1) * SHARD]
        deg_pad[k, :SHARD] = deg[k * SHARD:(k + 1) * SHARD]

    per_core = []
    for k in range(P):
        perm = [k] + [c for c in range(P) if c != k]
        xp = x_pad[perm].reshape(TBL, IN_DIM)
        degp = deg_pad[perm].reshape(P * NBLK, BLK).T   # [128, 392]
        per_core.append(dict(
            xp=np.ascontiguousarray(xp),
            degp=np.ascontiguousarray(degp),
            g1=np.ascontiguousarray(g1[k].reshape(T, BLK).T.astype(np.int32)),
            g23=np.ascontiguousarray(g23[k].reshape(T, BLK).T.astype(np.int32)),
            dstl=np.ascontiguousarray(dv[k].reshape(T, BLK).T),
        ))
    return per_core, C_BLK


# ------------------------------------------------------------- device build
def _build(C_BLK):
    T = NBLK * C_BLK
    NFULL = P * NBLK          # 392 blocks in the full node space

    nc = bacc.Bacc("TRN2", target_bir_lowering=False, debug=False,
                   enable_asserts=False, num_devices=P)

    xp_d = nc.dram_tensor("xp", [TBL, IN_DIM], F32, kind="ExternalInput").ap()
    degp_d = nc.dram_tensor("degp", [BLK, NFULL], F32, kind="ExternalInput").ap()
    g1_d = nc.dram_tensor("g1", [BLK, T], I32, kind="ExternalInput").ap()
    g23_d = nc.dram_tensor("g23", [BLK, T], I32, kind="ExternalInput").ap()
    dstl_d = nc.dram_tensor("dstl", [BLK, T], F32, kind="ExternalInput").ap()
    w_d = [nc.dram_tensor(f"w{i}", [d, HID if i < 3 else OUT_DIM], F32,
                          kind="ExternalInput").ap()
           for i, d in enumerate([IN_DIM, HID, HID, HID])]
    bt_d = [nc.dram_tensor(f"bt{i}", [BLK, HID if i < 3 else OUT_DIM], F32,
                           kind="ExternalInput").ap()
            for i in range(4)]
    iota_d = nc.dram_tensor("iota", [BLK, BLK], F32, kind="ExternalInput").ap()
    iden_d = nc.dram_tensor("iden", [BLK, BLK], F32, kind="ExternalInput").ap()
    out_d = nc.dram_tensor("probs", [PADS, OUT_DIM], F32, kind="ExternalOutput").ap()

    rg = [list(range(P))]

    with tile.TileContext(nc) as tc:
        with (
            tc.tile_pool(name="const", bufs=1) as cp,
            tc.tile_pool(name="xin", bufs=3) as xp_pool,
            tc.tile_pool(name="ht", bufs=3) as hp,
            tc.tile_pool(name="zt", bufs=3) as zp,
            tc.tile_pool(name="oh", bufs=6) as ohp,
            tc.tile_pool(name="msg", bufs=16) as mp,
            tc.tile_pool(name="fin", bufs=2) as fp,
            tc.tile_pool(name="pstp", bufs=2, space="PSUM") as pstp,
            tc.tile_pool(name="psacc", bufs=4, space="PSUM") as psacc,
            tc.tile_pool(name="dram", bufs=1, space="DRAM") as dp,
        ):
            # ---- constants into SBUF
            w_sb, bt_sb = [], []
            for i in range(4):
                wt = cp.tile(list(w_d[i].shape), F32, tag=f"w{i}", name=f"w{i}")
                nc.sync.dma_start(wt[:], w_d[i])
                w_sb.append(wt)
                bt = cp.tile(list(bt_d[i].shape), F32, tag=f"bt{i}", name=f"bt{i}")
                nc.sync.dma_start(bt[:], bt_d[i])
                bt_sb.append(bt)
            iota_sb = cp.tile([BLK, BLK], F32, tag="iota")
            nc.sync.dma_start(iota_sb[:], iota_d)
            iden_sb = cp.tile([BLK, BLK], F32, tag="iden")
            nc.sync.dma_start(iden_sb[:], iden_d)
            g1_sb = cp.tile([BLK, T], I32, tag="g1")
            nc.sync.dma_start(g1_sb[:], g1_d)
            g23_sb = cp.tile([BLK, T], I32, tag="g23")
            nc.sync.dma_start(g23_sb[:], g23_d)
            dstl_sb = cp.tile([BLK, T], F32, tag="dstl")
            nc.sync.dma_start(dstl_sb[:], dstl_d)

            deg_sb = cp.tile([BLK, NFULL], F32, tag="deg")
            nc.sync.dma_start(deg_sb[:], degp_d)
            dinv_sb = cp.tile([BLK, NFULL], F32, tag="dinv")
            nc.vector.reciprocal(dinv_sb[:], deg_sb[:])
            nc.scalar.activation(dinv_sb[:], dinv_sb[:],
                                 mybir.ActivationFunctionType.Sqrt)

            h_sb = [cp.tile([BLK, NBLK * HID], F32, tag=f"h{i}", name=f"h{i}")
                    for i in range(2)]
            zt_own = cp.tile([BLK, NBLK * HID], F32, tag="zt_own")

            def transform_block(src_ap, d_in, w_t, b, zdst):
                """z~_block = dinv[:,b] * (src_block @ W) -> zdst [128, HID]"""
                tp_ps = pstp.tile([d_in, BLK], F32, tag="tp", name="tp")
                nc.tensor.transpose(tp_ps[:], src_ap, iden_sb[:])
                hT = hp.tile([d_in, BLK], F32, tag="hT", name="hT")
                nc.vector.tensor_copy(hT[:], tp_ps[:])
                z_ps = psacc.tile([BLK, HID], F32, tag="acc", name="z_ps")
                nc.tensor.matmul(z_ps[:], hT[:], w_t[:], start=True, stop=True)
                nc.vector.tensor_scalar(zdst, z_ps[:], dinv_sb[:, b:b + 1],
                                        None, mybir.AluOpType.mult)

            def propagate(gidx_sb, table, h_nxt, b_t):
                for b in range(NBLK):
                    agg_ps = psacc.tile([BLK, HID], F32, tag="acc", name="agg_ps")
                    for c in range(C_BLK):
                        t = b * C_BLK + c
                        msg = mp.tile([BLK, HID], F32, tag="msg", name="msg")
                        nc.gpsimd.indirect_dma_start(
                            out=msg[:], out_offset=None, in_=table[:],
                            in_offset=bass.IndirectOffsetOnAxis(
                                ap=gidx_sb[:, t:t + 1], axis=0))
                        oh = ohp.tile([BLK, BLK], F32, tag="oh", name="oh")
                        nc.vector.tensor_scalar(
                            oh[:], iota_sb[:], dstl_sb[:, t:t + 1], None,
                            mybir.AluOpType.is_equal)
                        nc.tensor.matmul(agg_ps[:], oh[:], msg[:],
                                         start=(c == 0), stop=(c == C_BLK - 1))
                    sl = slice(b * HID, (b + 1) * HID)
                    tot = zp.tile([BLK, HID], F32, tag="tot", name="tot")
                    nc.vector.tensor_tensor(tot[:], agg_ps[:], zt_own[:, sl],
                                            mybir.AluOpType.add)
                    nc.vector.scalar_tensor_tensor(
                        h_nxt[:, sl], tot[:], dinv_sb[:, b:b + 1], b_t[:],
                        mybir.AluOpType.mult, mybir.AluOpType.add)
                    nc.scalar.activation(h_nxt[:, sl], h_nxt[:, sl],
                                         mybir.ActivationFunctionType.Relu)

            # ---------------- layer 1: full local table (x replicated)
            table1 = dp.tile([TBL, HID], F32, tag="tbl0")
            for b in range(NFULL):
                src_t = xp_pool.tile([BLK, IN_DIM], F32, tag="xb", name="xb")
                nc.sync.dma_start(src_t[:], xp_d[b * BLK:(b + 1) * BLK, :])
                zdst = zp.tile([BLK, HID], F32, tag="zd", name="zd")
                transform_block(src_t[:], IN_DIM, w_sb[0], b, zdst[:])
                nc.sync.dma_start(table1[b * BLK:(b + 1) * BLK, :], zdst[:])
                if b < NBLK:
                    nc.vector.tensor_copy(zt_own[:, b * HID:(b + 1) * HID],
                                          zdst[:])
            propagate(g1_sb, table1, h_sb[0], bt_sb[0])

            # ---------------- layers 2, 3: shard transform + AllGather
            for li in (1, 2):
                h_cur = h_sb[(li + 1) % 2]
                h_nxt = h_sb[li % 2]
                for b in range(NBLK):
                    transform_block(h_cur[:, b * HID:(b + 1) * HID], HID,
                                    w_sb[li], b,
                                    zt_own[:, b * HID:(b + 1) * HID])
                ag_in = dp.tile([PADS, HID], F32, tag=f"agin{li}",
                                name=f"agin{li}")
                nc.sync.dma_start(
                    ag_in[:].rearrange("(b p) f -> p b f", p=BLK),
                    zt_own[:].rearrange("p (b f) -> p b f", f=HID))
                table = dp.tile([TBL, HID], F32, tag=f"tbl{li}",
                                name=f"table{li}", addr_space="Shared")
                nc.gpsimd.collective_compute(
                    "AllGather", mybir.AluOpType.bypass, replica_groups=rg,
                    ins=[ag_in.opt()], outs=[table.opt()])
                propagate(g23_sb, table, h_nxt, bt_sb[li])

            # ---------------- readout + softmax
            h3 = h_sb[0]
            for b in range(NBLK):
                tp_ps = pstp.tile([HID, BLK], F32, tag="tp", name="tp")
                nc.tensor.transpose(tp_ps[:], h3[:, b * HID:(b + 1) * HID],
                                    iden_sb[:])
                hT = hp.tile([HID, BLK], F32, tag="hT", name="hT")
                nc.vector.tensor_copy(hT[:], tp_ps[:])
                o_ps = psacc.tile([BLK, OUT_DIM], F32, tag="acc", name="o_ps")
                nc.tensor.matmul(o_ps[:], hT[:], w_sb[3][:],
                                 start=True, stop=True)
                logit = fp.tile([BLK, OUT_DIM], F32, tag="logit", name="logit")
                nc.vector.tensor_tensor(logit[:], o_ps[:], bt_sb[3][:],
                                        mybir.AluOpType.add)
                nmx = fp.tile([BLK, 1], F32, tag="nmx", name="nmx")
                nc.vector.reduce_max(nmx[:], logit[:],
                                     axis=mybir.AxisListType.X, negate=True)
                ex = fp.tile([BLK, OUT_DIM], F32, tag="ex", name="ex")
                ssum = fp.tile([BLK, 1], F32, tag="ssum", name="ssum")
                nc.scalar.activation(ex[:], logit[:],
                                     mybir.ActivationFunctionType.Exp,
                                     bias=nmx[:], accum_out=ssum[:])
                rs = fp.tile([BLK, 1], F32, tag="rs", name="rs")
                nc.vector.reciprocal(rs[:], ssum[:])
                prob = fp.tile([BLK, OUT_DIM], F32, tag="prob", name="prob")
                nc.vector.tensor_scalar(prob[:], ex[:], rs[:], None,
                                        mybir.AluOpType.mult)
                nc.sync.dma_start(out_d[b * BLK:(b + 1) * BLK, :], prob[:])

    nc.compile()
    return nc


# ------------------------------------------------------------- entry point
_CACHE = {}


def _get_program(C_BLK):
    if C_BLK not in _CACHE:
        _CACHE[C_BLK] = _build(C_BLK)
    return _CACHE[C_BLK]


def kernel(x, edge_index, W1, b1, W2, b2, W3, b3, Wr, br, trace=False):
    per_core, C_BLK = _preprocess(x, edge_index)
    nc = _get_program(C_BLK)

    ws = [np.asarray(w, np.float32) for w in (W1, W2, W3, Wr)]
    bts = [np.tile(np.asarray(b, np.float32).reshape(1, -1), (BLK, 1))
           for b in (b1, b2, b3, br)]
    iota = np.tile(np.arange(BLK, dtype=np.float32), (BLK, 1))
    iden = np.eye(BLK, dtype=np.float32)

    in_maps = []
    for k in range(P):
        m = dict(per_core[k])
        for i in range(4):
            m[f"w{i}"] = ws[i]
            m[f"bt{i}"] = bts[i]
        m["iota"] = iota
        m["iden"] = iden
        in_maps.append(m)

    res = run_bass_kernel_spmd(nc, in_maps, core_ids=list(range(P)),
                               trace=trace)
    out = np.empty((N, OUT_DIM), np.float32)
    for k in range(P):
        out[k * SHARD:(k + 1) * SHARD] = res.results[k]["probs"][:SHARD]
    kernel.last_results = res
    return out


# revision 6
# speedup vs baseline: 1.0731x; 1.0731x over previous
"""GCN (3-layer + readout) on 8 Trainium2 NeuronCores.

Strategy (dst-node sharding, 1D graph parallel):
  - Nodes are sharded across 8 cores (6250/core, padded to 6272 = 49 blocks
    of 128).  Each core aggregates messages for the edges whose dst lands in
    its shard.
  - Per layer: transform z = h @ W (PE), scale rows by dinv = deg^-1/2 so
    table rows are dinv[src]*z[src].  Layer 1's table is computed fully
    locally by every core (x is replicated; each core gets a node
    permutation with its own shard first so the self-term slice is uniform
    across the SPMD program).  Layers 2/3 AllGather the shard tables.
  - Edge gathers: one indirect DMA (standard InstDMACopy dynamic-offset
    SWDGE path, int32 row ids, one row per partition) per 128-edge chunk.
  - Scatter-add on the TensorEngine: per chunk a one-hot
    onehot[e, d] = (dst_local[e] == d) is built with one DVE
    tensor_scalar(is_equal) against an iota row; psum[128d, 64] +=
    onehot^T @ msg accumulates the dst-block aggregation.
  - Self loops (PyG GCNConv implicit): the own-shard table slice is kept in
    SBUF and added to the block aggregate before the dst-side dinv scale,
    yielding exactly dinv^2 * z.
  - Host-side preprocessing is strictly index/metadata work (edge bucketing,
    padding, degree counting, node permutation); all float math runs on
    device.
"""

import numpy as np

from concourse import bacc, bass, mybir, tile
from concourse.bass_utils import run_bass_kernel_spmd

# ---------------------------------------------------------------- constants
P = 8                      # cores
N = 50000                  # nodes
IN_DIM = 128
HID = 64
OUT_DIM = 10
BLK = 128

F32 = mybir.dt.float32
I32 = mybir.dt.int32


def _derived():
    shard = N // P
    nblk = (shard + BLK - 1) // BLK
    pads = nblk * BLK
    tbl = P * pads
    return shard, nblk, pads, tbl


def _refresh_dims():
    global SHARD, NBLK, PADS, TBL
    SHARD, NBLK, PADS, TBL = _derived()


SHARD, NBLK, PADS, TBL = _derived()


# ------------------------------------------------------------- host prep
def _preprocess(x, edge_index):
    """Bucket edges into per-(core, dst-block) 128-edge chunks."""
    x = np.asarray(x, np.float32)
    ei = np.asarray(edge_index, np.int64)
    src, dst = ei[0], ei[1]

    deg = np.bincount(dst, minlength=N).astype(np.float32) + 1.0

    owner = dst // SHARD
    blk = (dst - owner * SHARD) // BLK
    dstl = ((dst - owner * SHARD) % BLK).astype(np.float32)
    s_own = src // SHARD
    s_loc = src % SHARD

    # chunk slots
    gid = owner * NBLK + blk
    order = np.argsort(gid, kind="stable")
    gid_s = gid[order]
    counts = np.bincount(gid_s, minlength=P * NBLK)
    starts = np.concatenate([[0], np.cumsum(counts)[:-1]])
    pos = np.arange(gid_s.size) - starts[gid_s]

    C_BLK = int(np.ceil(counts.max() / BLK))
    T = NBLK * C_BLK

    own_s = gid_s // NBLK
    blk_s = gid_s % NBLK
    slot = blk_s * C_BLK * BLK + pos          # slot within the core's stream

    # AG-table row (rank-ordered layout, layers 2/3)
    row23 = (s_own * PADS + s_loc)[order]
    dstl_s = dstl[order]

    g23 = np.zeros((P, T * BLK), np.int32)
    dv = np.full((P, T * BLK), -1.0, np.float32)
    flat = own_s * (T * BLK) + slot
    g23.reshape(-1)[flat] = row23.astype(np.int32)
    dv.reshape(-1)[flat] = dstl_s

    # layer-1 table row: per-core permuted layout, own shard first
    g1 = np.zeros((P, T * BLK), np.int32)
    s_own_s = s_own[order]
    s_loc_s = s_loc[order]
    for k in range(P):
        sel = own_s == k
        so = s_own_s[sel]
        # position of shard `so` in core k's permuted order [k, 0,1,..(!k)..,7]
        rank = np.where(so == k, 0, 1 + so - (so > k))
        g1.reshape(-1)[flat[sel]] = (rank * PADS + s_loc_s[sel]).astype(np.int32)

    x_pad = np.zeros((P, PADS, IN_DIM), np.float32)
    deg_pad = np.ones((P, PADS), np.float32)
    for k in range(P):
        x_pad[k, :SHARD] = x[k * SHARD:(k + 1) * SHARD]
        deg_pad[k, :SHARD] = deg[k * SHARD:(k + 1) * SHARD]

    per_core = []
    for k in range(P):
        perm = [k] + [c for c in range(P) if c != k]
        xp = x_pad[perm].reshape(TBL, IN_DIM)
        degp = deg_pad[perm].reshape(P * NBLK, BLK).T   # [128, 392]
        per_core.append(dict(
            xp=np.ascontiguousarray(xp),
            degp=np.ascontiguousarray(degp),
            g1=np.ascontiguousarray(g1[k].reshape(T, BLK).T.astype(np.int32)),
            g23=np.ascontiguousarray(g23[k].reshape(T, BLK).T.astype(np.int32)),
            dstl=np.ascontiguousarray(dv[k].reshape(T, BLK).T),
        ))
    return per_core, C_BLK


# ------------------------------------------------------------- device build
def _build(C_BLK):
    T = NBLK * C_BLK
    NFULL = P * NBLK          # 392 blocks in the full node space

    nc = bacc.Bacc("TRN2", target_bir_lowering=False, debug=False,
                   enable_asserts=False, num_devices=P,
                   dynamic_dma_scratch_size=49152)

    xp_d = nc.dram_tensor("xp", [TBL, IN_DIM], F32, kind="ExternalInput").ap()
    degp_d = nc.dram_tensor("degp", [BLK, NFULL], F32, kind="ExternalInput").ap()
    g1_d = nc.dram_tensor("g1", [BLK, T], I32, kind="ExternalInput").ap()
    g23_d = nc.dram_tensor("g23", [BLK, T], I32, kind="ExternalInput").ap()
    dstl_d = nc.dram_tensor("dstl", [BLK, T], F32, kind="ExternalInput").ap()
    w_d = [nc.dram_tensor(f"w{i}", [d, HID if i < 3 else OUT_DIM], F32,
                          kind="ExternalInput").ap()
           for i, d in enumerate([IN_DIM, HID, HID, HID])]
    bt_d = [nc.dram_tensor(f"bt{i}", [BLK, HID if i < 3 else OUT_DIM], F32,
                           kind="ExternalInput").ap()
            for i in range(4)]
    iota_d = nc.dram_tensor("iota", [BLK, BLK], F32, kind="ExternalInput").ap()
    iden_d = nc.dram_tensor("iden", [BLK, BLK], F32, kind="ExternalInput").ap()
    out_d = nc.dram_tensor("probs", [PADS, OUT_DIM], F32, kind="ExternalOutput").ap()

    rg = [list(range(P))]

    with tile.TileContext(nc) as tc:
        with (
            tc.tile_pool(name="const", bufs=1) as cp,
            tc.tile_pool(name="xin", bufs=3) as xp_pool,
            tc.tile_pool(name="ht", bufs=3) as hp,
            tc.tile_pool(name="zt", bufs=3) as zp,
            tc.tile_pool(name="oh", bufs=6) as ohp,
            tc.tile_pool(name="msg", bufs=16) as mp,
            tc.tile_pool(name="fin", bufs=2) as fp,
            tc.tile_pool(name="pstp", bufs=2, space="PSUM") as pstp,
            tc.tile_pool(name="psacc", bufs=4, space="PSUM") as psacc,
            tc.tile_pool(name="dram", bufs=1, space="DRAM") as dp,
        ):
            # ---- constants into SBUF
            w_sb, bt_sb = [], []
            for i in range(4):
                wt = cp.tile(list(w_d[i].shape), F32, tag=f"w{i}", name=f"w{i}")
                nc.sync.dma_start(wt[:], w_d[i])
                w_sb.append(wt)
                bt = cp.tile(list(bt_d[i].shape), F32, tag=f"bt{i}", name=f"bt{i}")
                nc.sync.dma_start(bt[:], bt_d[i])
                bt_sb.append(bt)
            iota_sb = cp.tile([BLK, BLK], F32, tag="iota")
            nc.sync.dma_start(iota_sb[:], iota_d)
            iden_sb = cp.tile([BLK, BLK], F32, tag="iden")
            nc.sync.dma_start(iden_sb[:], iden_d)
            g1_sb = cp.tile([BLK, T], I32, tag="g1")
            nc.sync.dma_start(g1_sb[:], g1_d)
            g23_sb = cp.tile([BLK, T], I32, tag="g23")
            nc.sync.dma_start(g23_sb[:], g23_d)
            dstl_sb = cp.tile([BLK, T], F32, tag="dstl")
            nc.sync.dma_start(dstl_sb[:], dstl_d)

            deg_sb = cp.tile([BLK, NFULL], F32, tag="deg")
            nc.sync.dma_start(deg_sb[:], degp_d)
            dinv_sb = cp.tile([BLK, NFULL], F32, tag="dinv")
            nc.vector.reciprocal(dinv_sb[:], deg_sb[:])
            nc.scalar.activation(dinv_sb[:], dinv_sb[:],
                                 mybir.ActivationFunctionType.Sqrt)

            h_sb = [cp.tile([BLK, NBLK * HID], F32, tag=f"h{i}", name=f"h{i}")
                    for i in range(2)]
            zt_own = cp.tile([BLK, NBLK * HID], F32, tag="zt_own")

            def transform_block(src_ap, d_in, w_t, b, zdst):
                """z~_block = dinv[:,b] * (src_block @ W) -> zdst [128, HID]"""
                tp_ps = pstp.tile([d_in, BLK], F32, tag="tp", name="tp")
                nc.tensor.transpose(tp_ps[:], src_ap, iden_sb[:])
                hT = hp.tile([d_in, BLK], F32, tag="hT", name="hT")
                nc.vector.tensor_copy(hT[:], tp_ps[:])
                z_ps = psacc.tile([BLK, HID], F32, tag="acc", name="z_ps")
                nc.tensor.matmul(z_ps[:], hT[:], w_t[:], start=True, stop=True)
                nc.vector.tensor_scalar(zdst, z_ps[:], dinv_sb[:, b:b + 1],
                                        None, mybir.AluOpType.mult)

            def propagate(gidx_sb, table, h_nxt, b_t):
                for b in range(NBLK):
                    agg_ps = psacc.tile([BLK, HID], F32, tag="acc", name="agg_ps")
                    for c in range(C_BLK):
                        t = b * C_BLK + c
                        msg = mp.tile([BLK, HID], F32, tag="msg", name="msg")
                        nc.gpsimd.indirect_dma_start(
                            out=msg[:], out_offset=None, in_=table[:],
                            in_offset=bass.IndirectOffsetOnAxis(
                                ap=gidx_sb[:, t:t + 1], axis=0))
                        oh = ohp.tile([BLK, BLK], F32, tag="oh", name="oh")
                        nc.vector.tensor_scalar(
                            oh[:], iota_sb[:], dstl_sb[:, t:t + 1], None,
                            mybir.AluOpType.is_equal)
                        nc.tensor.matmul(agg_ps[:], oh[:], msg[:],
                                         start=(c == 0), stop=(c == C_BLK - 1))
                    sl = slice(b * HID, (b + 1) * HID)
                    tot = zp.tile([BLK, HID], F32, tag="tot", name="tot")
                    nc.vector.tensor_tensor(tot[:], agg_ps[:], zt_own[:, sl],
                                            mybir.AluOpType.add)
                    nc.vector.scalar_tensor_tensor(
                        h_nxt[:, sl], tot[:], dinv_sb[:, b:b + 1], b_t[:],
                        mybir.AluOpType.mult, mybir.AluOpType.add)
                    nc.scalar.activation(h_nxt[:, sl], h_nxt[:, sl],
                                         mybir.ActivationFunctionType.Relu)

            # ---------------- layer 1: full local table (x replicated)
            # batched 8-block staging keeps the sync engine off the critical
            # path (one 512KB load + one 256KB store per 8 blocks)
            table1 = dp.tile([TBL, HID], F32, tag="tbl0")
            GB = 8
            for g in range(NFULL // GB):
                xg = xp_pool.tile([BLK, GB * IN_DIM], F32, tag="xb", name="xb")
                nc.sync.dma_start(
                    xg[:].rearrange("p (j f) -> p j f", f=IN_DIM),
                    xp_d[g * GB * BLK:(g + 1) * GB * BLK, :].rearrange(
                        "(j p) f -> p j f", p=BLK))
                zg = zp.tile([BLK, GB * HID], F32, tag="zd", name="zd")
                for j in range(GB):
                    b = g * GB + j
                    transform_block(xg[:, j * IN_DIM:(j + 1) * IN_DIM],
                                    IN_DIM, w_sb[0], b,
                                    zg[:, j * HID:(j + 1) * HID])
                    if b < NBLK:
                        nc.vector.tensor_copy(
                            zt_own[:, b * HID:(b + 1) * HID],
                            zg[:, j * HID:(j + 1) * HID])
                nc.sync.dma_start(
                    table1[g * GB * BLK:(g + 1) * GB * BLK, :].rearrange(
                        "(j p) f -> p j f", p=BLK),
                    zg[:].rearrange("p (j f) -> p j f", f=HID))
            propagate(g1_sb, table1, h_sb[0], bt_sb[0])

            # ---------------- layers 2, 3: shard transform + AllGather
            for li in (1, 2):
                h_cur = h_sb[(li + 1) % 2]
                h_nxt = h_sb[li % 2]
                for b in range(NBLK):
                    transform_block(h_cur[:, b * HID:(b + 1) * HID], HID,
                                    w_sb[li], b,
                                    zt_own[:, b * HID:(b + 1) * HID])
                ag_in = dp.tile([PADS, HID], F32, tag=f"agin{li}",
                                name=f"agin{li}")
                nc.sync.dma_start(
                    ag_in[:].rearrange("(b p) f -> p b f", p=BLK),
                    zt_own[:].rearrange("p (b f) -> p b f", f=HID))
                table = dp.tile([TBL, HID], F32, tag=f"tbl{li}",
                                name=f"table{li}", addr_space="Shared")
                nc.gpsimd.collective_compute(
                    "AllGather", mybir.AluOpType.bypass, replica_groups=rg,
                    ins=[ag_in.opt()], outs=[table.opt()])
                propagate(g23_sb, table, h_nxt, bt_sb[li])

            # ---------------- readout + softmax
            h3 = h_sb[0]
            for b in range(NBLK):
                tp_ps = pstp.tile([HID, BLK], F32, tag="tp", name="tp")
                nc.tensor.transpose(tp_ps[:], h3[:, b * HID:(b + 1) * HID],
                                    iden_sb[:])
                hT = hp.tile([HID, BLK], F32, tag="hT", name="hT")
                nc.vector.tensor_copy(hT[:], tp_ps[:])
                o_ps = psacc.tile([BLK, OUT_DIM], F32, tag="acc", name="o_ps")
                nc.tensor.matmul(o_ps[:], hT[:], w_sb[3][:],
                                 start=True, stop=True)
                logit = fp.tile([BLK, OUT_DIM], F32, tag="logit", name="logit")
                nc.vector.tensor_tensor(logit[:], o_ps[:], bt_sb[3][:],
                                        mybir.AluOpType.add)
                nmx = fp.tile([BLK, 1], F32, tag="nmx", name="nmx")
                nc.vector.reduce_max(nmx[:], logit[:],
                                     axis=mybir.AxisListType.X, negate=True)
                ex = fp.tile([BLK, OUT_DIM], F32, tag="ex", name="ex")
                ssum = fp.tile([BLK, 1], F32, tag="ssum", name="ssum")
                nc.scalar.activation(ex[:], logit[:],
                                     mybir.ActivationFunctionType.Exp,
                                     bias=nmx[:], accum_out=ssum[:])
                rs = fp.tile([BLK, 1], F32, tag="rs", name="rs")
                nc.vector.reciprocal(rs[:], ssum[:])
                prob = fp.tile([BLK, OUT_DIM], F32, tag="prob", name="prob")
                nc.vector.tensor_scalar(prob[:], ex[:], rs[:], None,
                                        mybir.AluOpType.mult)
                nc.sync.dma_start(out_d[b * BLK:(b + 1) * BLK, :], prob[:])

    nc.compile()
    return nc


# ------------------------------------------------------------- entry point
_CACHE = {}


def _get_program(C_BLK):
    if C_BLK not in _CACHE:
        _CACHE[C_BLK] = _build(C_BLK)
    return _CACHE[C_BLK]


def kernel(x, edge_index, W1, b1, W2, b2, W3, b3, Wr, br, trace=False):
    per_core, C_BLK = _preprocess(x, edge_index)
    nc = _get_program(C_BLK)

    ws = [np.asarray(w, np.float32) for w in (W1, W2, W3, Wr)]
    bts = [np.tile(np.asarray(b, np.float32).reshape(1, -1), (BLK, 1))
           for b in (b1, b2, b3, br)]
    iota = np.tile(np.arange(BLK, dtype=np.float32), (BLK, 1))
    iden = np.eye(BLK, dtype=np.float32)

    in_maps = []
    for k in range(P):
        m = dict(per_core[k])
        for i in range(4):
            m[f"w{i}"] = ws[i]
            m[f"bt{i}"] = bts[i]
        m["iota"] = iota
        m["iden"] = iden
        in_maps.append(m)

    res = run_bass_kernel_spmd(nc, in_maps, core_ids=list(range(P)),
                               trace=trace)
    out = np.empty((N, OUT_DIM), np.float32)
    for k in range(P):
        out[k * SHARD:(k + 1) * SHARD] = res.results[k]["probs"][:SHARD]
    kernel.last_results = res
    return out


# revision 13
# speedup vs baseline: 1.1432x; 1.0654x over previous
"""GCN (3-layer + readout) on 8 Trainium2 NeuronCores.

Strategy (dst-node sharding, 1D graph parallel):
  - Nodes are sharded across 8 cores (6250/core, padded to 6272 = 49 blocks
    of 128).  Each core aggregates messages for the edges whose dst lands in
    its shard.
  - Per layer: transform z = h @ W (PE), scale rows by dinv = deg^-1/2 so
    table rows are dinv[src]*z[src].  Layer 1's table is computed fully
    locally by every core (x is replicated; each core gets a node
    permutation with its own shard first so the self-term slice is uniform
    across the SPMD program).  Layers 2/3 AllGather the shard tables.
  - Edge gathers: one indirect DMA (standard InstDMACopy dynamic-offset
    SWDGE path, int32 row ids, one row per partition) per 128-edge chunk.
  - Scatter-add on the TensorEngine: per chunk a one-hot
    onehot[e, d] = (dst_local[e] == d) is built with one DVE
    tensor_scalar(is_equal) against an iota row; psum[128d, 64] +=
    onehot^T @ msg accumulates the dst-block aggregation.
  - Self loops (PyG GCNConv implicit): the own-shard table slice is kept in
    SBUF and added to the block aggregate before the dst-side dinv scale,
    yielding exactly dinv^2 * z.
  - Host-side preprocessing is strictly index/metadata work (edge bucketing,
    padding, degree counting, node permutation); all float math runs on
    device.
"""

import numpy as np

from concourse import bacc, bass, mybir, tile
from concourse.bass_utils import run_bass_kernel_spmd

# ---------------------------------------------------------------- constants
P = 8                      # cores
N = 50000                  # nodes
IN_DIM = 128
HID = 64
OUT_DIM = 10
BLK = 128

F32 = mybir.dt.float32
I32 = mybir.dt.int32


def _derived():
    shard = N // P
    nblk = (shard + BLK - 1) // BLK
    pads = nblk * BLK
    tbl = P * pads
    return shard, nblk, pads, tbl


def _refresh_dims():
    global SHARD, NBLK, PADS, TBL
    SHARD, NBLK, PADS, TBL = _derived()


SHARD, NBLK, PADS, TBL = _derived()


# ------------------------------------------------------------- host prep
def _preprocess(x, edge_index):
    """Bucket edges into per-(core, dst-block) 128-edge chunks."""
    x = np.asarray(x, np.float32)
    ei = np.asarray(edge_index, np.int64)
    src, dst = ei[0], ei[1]

    deg = np.bincount(dst, minlength=N).astype(np.float32) + 1.0

    owner = dst // SHARD
    blk = (dst - owner * SHARD) // BLK
    dstl = ((dst - owner * SHARD) % BLK).astype(np.float32)
    s_own = src // SHARD
    s_loc = src % SHARD

    # chunk slots
    gid = owner * NBLK + blk
    order = np.argsort(gid, kind="stable")
    gid_s = gid[order]
    counts = np.bincount(gid_s, minlength=P * NBLK)
    starts = np.concatenate([[0], np.cumsum(counts)[:-1]])
    pos = np.arange(gid_s.size) - starts[gid_s]

    # per-block chunk count: max over cores (program must be core-uniform)
    C_arr = np.maximum(
        np.ceil(counts.reshape(P, NBLK).max(axis=0) / BLK).astype(np.int64), 1)
    base = np.concatenate([[0], np.cumsum(C_arr)[:-1]])
    T = int(C_arr.sum())

    own_s = gid_s // NBLK
    blk_s = gid_s % NBLK
    slot = base[blk_s] * BLK + pos            # slot within the core's stream

    # AG-table row (rank-ordered layout, layers 2/3)
    row23 = (s_own * PADS + s_loc)[order]
    dstl_s = dstl[order]

    g23 = np.zeros((P, T * BLK), np.int32)
    dv = np.full((P, T * BLK), -1.0, np.float32)
    flat = own_s * (T * BLK) + slot
    g23.reshape(-1)[flat] = row23.astype(np.int32)
    dv.reshape(-1)[flat] = dstl_s

    # layer-1 table row: per-core permuted layout, own shard first
    g1 = np.zeros((P, T * BLK), np.int32)
    s_own_s = s_own[order]
    s_loc_s = s_loc[order]
    for k in range(P):
        sel = own_s == k
        so = s_own_s[sel]
        # position of shard `so` in core k's permuted order [k, 0,1,..(!k)..,7]
        rank = np.where(so == k, 0, 1 + so - (so > k))
        g1.reshape(-1)[flat[sel]] = (rank * PADS + s_loc_s[sel]).astype(np.int32)

    x_pad = np.zeros((P, PADS, IN_DIM), np.float32)
    deg_pad = np.ones((P, PADS), np.float32)
    for k in range(P):
        x_pad[k, :SHARD] = x[k * SHARD:(k + 1) * SHARD]
        deg_pad[k, :SHARD] = deg[k * SHARD:(k + 1) * SHARD]

    per_core = []
    for k in range(P):
        perm = [k] + [c for c in range(P) if c != k]
        xp = x_pad[perm].reshape(TBL, IN_DIM)
        degp = deg_pad[perm].reshape(P * NBLK, BLK).T   # [128, 392]
        per_core.append(dict(
            xp=np.ascontiguousarray(xp),
            degp=np.ascontiguousarray(degp),
            g1=np.ascontiguousarray(g1[k].reshape(T, BLK).T.astype(np.int32)),
            g23=np.ascontiguousarray(g23[k].reshape(T, BLK).T.astype(np.int32)),
            dstl=np.ascontiguousarray(dv[k].reshape(T, BLK).T),
        ))
    return per_core, tuple(int(c) for c in C_arr)


# ------------------------------------------------------------- device build
def _build(C_arr):
    T = int(sum(C_arr))
    c_base = [0]
    for c in C_arr[:-1]:
        c_base.append(c_base[-1] + c)
    NFULL = P * NBLK          # 392 blocks in the full node space

    nc = bacc.Bacc("TRN2", target_bir_lowering=False, debug=False,
                   enable_asserts=False, num_devices=P,
                   dynamic_dma_scratch_size=65536)

    xp_d = nc.dram_tensor("xp", [TBL, IN_DIM], F32, kind="ExternalInput").ap()
    degp_d = nc.dram_tensor("degp", [BLK, NFULL], F32, kind="ExternalInput").ap()
    g1_d = nc.dram_tensor("g1", [BLK, T], I32, kind="ExternalInput").ap()
    g23_d = nc.dram_tensor("g23", [BLK, T], I32, kind="ExternalInput").ap()
    dstl_d = nc.dram_tensor("dstl", [BLK, T], F32, kind="ExternalInput").ap()
    w_d = [nc.dram_tensor(f"w{i}", [d, HID if i < 3 else OUT_DIM], F32,
                          kind="ExternalInput").ap()
           for i, d in enumerate([IN_DIM, HID, HID, HID])]
    bt_d = [nc.dram_tensor(f"bt{i}", [BLK, HID if i < 3 else OUT_DIM], F32,
                           kind="ExternalInput").ap()
            for i in range(4)]
    iota_d = nc.dram_tensor("iota", [BLK, BLK], F32, kind="ExternalInput").ap()
    iden_d = nc.dram_tensor("iden", [BLK, BLK], F32, kind="ExternalInput").ap()
    out_d = nc.dram_tensor("probs", [PADS, OUT_DIM], F32, kind="ExternalOutput").ap()

    rg = [list(range(P))]

    with tile.TileContext(nc) as tc:
        with (
            tc.tile_pool(name="const", bufs=1) as cp,
            tc.tile_pool(name="xin", bufs=3) as xp_pool,
            tc.tile_pool(name="ht", bufs=3) as hp,
            tc.tile_pool(name="zt", bufs=3) as zp,
            tc.tile_pool(name="oh", bufs=12) as ohp,
            tc.tile_pool(name="msg", bufs=32) as mp,
            tc.tile_pool(name="fin", bufs=2) as fp,
            tc.tile_pool(name="pstp", bufs=2, space="PSUM") as pstp,
            tc.tile_pool(name="psacc", bufs=4, space="PSUM") as psacc,
            tc.tile_pool(name="dram", bufs=1, space="DRAM") as dp,
        ):
            # ---- constants into SBUF
            w_sb, bt_sb = [], []
            for i in range(4):
                wt = cp.tile(list(w_d[i].shape), F32, tag=f"w{i}", name=f"w{i}")
                nc.sync.dma_start(wt[:], w_d[i])
                w_sb.append(wt)
                bt = cp.tile(list(bt_d[i].shape), F32, tag=f"bt{i}", name=f"bt{i}")
                nc.sync.dma_start(bt[:], bt_d[i])
                bt_sb.append(bt)
            iota_sb = cp.tile([BLK, BLK], F32, tag="iota")
            nc.sync.dma_start(iota_sb[:], iota_d)
            iden_sb = cp.tile([BLK, BLK], F32, tag="iden")
            nc.sync.dma_start(iden_sb[:], iden_d)
            g1_sb = cp.tile([BLK, T], I32, tag="g1")
            nc.sync.dma_start(g1_sb[:], g1_d)
            g23_sb = cp.tile([BLK, T], I32, tag="g23")
            nc.sync.dma_start(g23_sb[:], g23_d)
            dstl_sb = cp.tile([BLK, T], F32, tag="dstl")
            nc.sync.dma_start(dstl_sb[:], dstl_d)

            deg_sb = cp.tile([BLK, NFULL], F32, tag="deg")
            nc.sync.dma_start(deg_sb[:], degp_d)
            dinv_sb = cp.tile([BLK, NFULL], F32, tag="dinv")
            nc.vector.reciprocal(dinv_sb[:], deg_sb[:])
            nc.scalar.activation(dinv_sb[:], dinv_sb[:],
                                 mybir.ActivationFunctionType.Sqrt)

            h_sb = [cp.tile([BLK, NBLK * HID], F32, tag=f"h{i}", name=f"h{i}")
                    for i in range(2)]
            zt_own = cp.tile([BLK, NBLK * HID], F32, tag="zt_own")

            def transform_block(src_ap, d_in, w_t, b, zdst):
                """z~_block = dinv[:,b] * (src_block @ W) -> zdst [128, HID]"""
                tp_ps = pstp.tile([d_in, BLK], F32, tag="tp", name="tp")
                nc.tensor.transpose(tp_ps[:], src_ap, iden_sb[:])
                hT = hp.tile([d_in, BLK], F32, tag="hT", name="hT")
                nc.vector.tensor_copy(hT[:], tp_ps[:])
                z_ps = psacc.tile([BLK, HID], F32, tag="acc", name="z_ps")
                nc.tensor.matmul(z_ps[:], hT[:], w_t[:], start=True, stop=True)
                nc.vector.tensor_scalar(zdst, z_ps[:], dinv_sb[:, b:b + 1],
                                        None, mybir.AluOpType.mult)

            def readout_block(h_ap, b):
                tp_ps = pstp.tile([HID, BLK], F32, tag="tp", name="tp")
                nc.tensor.transpose(tp_ps[:], h_ap, iden_sb[:])
                hT = hp.tile([HID, BLK], F32, tag="hT", name="hT")
                nc.vector.tensor_copy(hT[:], tp_ps[:])
                o_ps = psacc.tile([BLK, OUT_DIM], F32, tag="acc", name="o_ps")
                nc.tensor.matmul(o_ps[:], hT[:], w_sb[3][:],
                                 start=True, stop=True)
                logit = fp.tile([BLK, OUT_DIM], F32, tag="logit", name="logit")
                nc.vector.tensor_tensor(logit[:], o_ps[:], bt_sb[3][:],
                                        mybir.AluOpType.add)
                nmx = fp.tile([BLK, 1], F32, tag="nmx", name="nmx")
                nc.vector.reduce_max(nmx[:], logit[:],
                                     axis=mybir.AxisListType.X, negate=True)
                ex = fp.tile([BLK, OUT_DIM], F32, tag="ex", name="ex")
                ssum = fp.tile([BLK, 1], F32, tag="ssum", name="ssum")
                nc.scalar.activation(ex[:], logit[:],
                                     mybir.ActivationFunctionType.Exp,
                                     bias=nmx[:], accum_out=ssum[:])
                rs = fp.tile([BLK, 1], F32, tag="rs", name="rs")
                nc.vector.reciprocal(rs[:], ssum[:])
                prob = fp.tile([BLK, OUT_DIM], F32, tag="prob", name="prob")
                nc.vector.tensor_scalar(prob[:], ex[:], rs[:], None,
                                        mybir.AluOpType.mult)
                nc.sync.dma_start(out_d[b * BLK:(b + 1) * BLK, :], prob[:])

            def propagate(gidx_sb, table, h_nxt, b_t, readout=False):
                for b in range(NBLK):
                    C_b = C_arr[b]
                    agg_ps = psacc.tile([BLK, HID], F32, tag="acc", name="agg_ps")
                    for c in range(C_b):
                        t = c_base[b] + c
                        msg = mp.tile([BLK, HID], F32, tag="msg", name="msg")
                        nc.gpsimd.indirect_dma_start(
                            out=msg[:], out_offset=None, in_=table[:],
                            in_offset=bass.IndirectOffsetOnAxis(
                                ap=gidx_sb[:, t:t + 1], axis=0))
                        oh = ohp.tile([BLK, BLK], F32, tag="oh", name="oh")
                        nc.vector.tensor_scalar(
                            oh[:], iota_sb[:], dstl_sb[:, t:t + 1], None,
                            mybir.AluOpType.is_equal)
                        nc.tensor.matmul(agg_ps[:], oh[:], msg[:],
                                         start=(c == 0), stop=(c == C_b - 1))
                    sl = slice(b * HID, (b + 1) * HID)
                    tot = zp.tile([BLK, HID], F32, tag="tot", name="tot")
                    nc.vector.tensor_tensor(tot[:], agg_ps[:], zt_own[:, sl],
                                            mybir.AluOpType.add)
                    nc.vector.scalar_tensor_tensor(
                        h_nxt[:, sl], tot[:], dinv_sb[:, b:b + 1], b_t[:],
                        mybir.AluOpType.mult, mybir.AluOpType.add)
                    nc.scalar.activation(h_nxt[:, sl], h_nxt[:, sl],
                                         mybir.ActivationFunctionType.Relu)
                    if readout:
                        readout_block(h_nxt[:, sl], b)

            # ---------------- layer 1: full local table (x replicated)
            # batched 8-block staging keeps the sync engine off the critical
            # path (one 512KB load + one 256KB store per 8 blocks)
            table1 = dp.tile([TBL, HID], F32, tag="tbl0")
            GB = 8
            for g in range(NFULL // GB):
                xg = xp_pool.tile([BLK, GB * IN_DIM], F32, tag="xb", name="xb")
                nc.sync.dma_start(
                    xg[:].rearrange("p (j f) -> p j f", f=IN_DIM),
                    xp_d[g * GB * BLK:(g + 1) * GB * BLK, :].rearrange(
                        "(j p) f -> p j f", p=BLK))
                zg = zp.tile([BLK, GB * HID], F32, tag="zd", name="zd")
                for j in range(GB):
                    b = g * GB + j
                    transform_block(xg[:, j * IN_DIM:(j + 1) * IN_DIM],
                                    IN_DIM, w_sb[0], b,
                                    zg[:, j * HID:(j + 1) * HID])
                    if b < NBLK:
                        nc.vector.tensor_copy(
                            zt_own[:, b * HID:(b + 1) * HID],
                            zg[:, j * HID:(j + 1) * HID])
                nc.sync.dma_start(
                    table1[g * GB * BLK:(g + 1) * GB * BLK, :].rearrange(
                        "(j p) f -> p j f", p=BLK),
                    zg[:].rearrange("p (j f) -> p j f", f=HID))
            propagate(g1_sb, table1, h_sb[0], bt_sb[0])

            # ---------------- layers 2, 3: shard transform + AllGather
            for li in (1, 2):
                h_cur = h_sb[(li + 1) % 2]
                h_nxt = h_sb[li % 2]
                for b in range(NBLK):
                    transform_block(h_cur[:, b * HID:(b + 1) * HID], HID,
                                    w_sb[li], b,
                                    zt_own[:, b * HID:(b + 1) * HID])
                ag_in = dp.tile([PADS, HID], F32, tag=f"agin{li}",
                                name=f"agin{li}")
                nc.sync.dma_start(
                    ag_in[:].rearrange("(b p) f -> p b f", p=BLK),
                    zt_own[:].rearrange("p (b f) -> p b f", f=HID))
                table = dp.tile([TBL, HID], F32, tag=f"tbl{li}",
                                name=f"table{li}", addr_space="Shared")
                nc.gpsimd.collective_compute(
                    "AllGather", mybir.AluOpType.bypass, replica_groups=rg,
                    ins=[ag_in.opt()], outs=[table.opt()])
                propagate(g23_sb, table, h_nxt, bt_sb[li], readout=(li == 2))

    nc.compile()
    return nc


# ------------------------------------------------------------- entry point
_CACHE = {}


def _get_program(C_arr):
    if C_arr not in _CACHE:
        _CACHE[C_arr] = _build(C_arr)
    return _CACHE[C_arr]


def kernel(x, edge_index, W1, b1, W2, b2, W3, b3, Wr, br, trace=False):
    per_core, C_arr = _preprocess(x, edge_index)
    nc = _get_program(C_arr)

    ws = [np.asarray(w, np.float32) for w in (W1, W2, W3, Wr)]
    bts = [np.tile(np.asarray(b, np.float32).reshape(1, -1), (BLK, 1))
           for b in (b1, b2, b3, br)]
    iota = np.tile(np.arange(BLK, dtype=np.float32), (BLK, 1))
    iden = np.eye(BLK, dtype=np.float32)

    in_maps = []
    for k in range(P):
        m = dict(per_core[k])
        for i in range(4):
            m[f"w{i}"] = ws[i]
            m[f"bt{i}"] = bts[i]
        m["iota"] = iota
        m["iden"] = iden
        in_maps.append(m)

    res = run_bass_kernel_spmd(nc, in_maps, core_ids=list(range(P)),
                               trace=trace)
    out = np.empty((N, OUT_DIM), np.float32)
    for k in range(P):
        out[k * SHARD:(k + 1) * SHARD] = res.results[k]["probs"][:SHARD]
    kernel.last_results = res
    return out


# revision 16
# speedup vs baseline: 1.1557x; 1.0109x over previous
"""GCN (3-layer + readout) on 8 Trainium2 NeuronCores.

Strategy (dst-node sharding, 1D graph parallel):
  - Nodes are sharded across 8 cores (6250/core, padded to 6272 = 49 blocks
    of 128).  Each core aggregates messages for the edges whose dst lands in
    its shard.
  - Per layer: transform z = h @ W (PE), scale rows by dinv = deg^-1/2 so
    table rows are dinv[src]*z[src].  Layer 1's table is computed fully
    locally by every core (x is replicated; each core gets a node
    permutation with its own shard first so the self-term slice is uniform
    across the SPMD program).  Layers 2/3 AllGather the shard tables.
  - Edge gathers: one indirect DMA (standard InstDMACopy dynamic-offset
    SWDGE path, int32 row ids, one row per partition) per 128-edge chunk.
  - Scatter-add on the TensorEngine: per chunk a one-hot
    onehot[e, d] = (dst_local[e] == d) is built with one DVE
    tensor_scalar(is_equal) against an iota row; psum[128d, 64] +=
    onehot^T @ msg accumulates the dst-block aggregation.
  - Self loops (PyG GCNConv implicit): the own-shard table slice is kept in
    SBUF and added to the block aggregate before the dst-side dinv scale,
    yielding exactly dinv^2 * z.
  - Host-side preprocessing is strictly index/metadata work (edge bucketing,
    padding, degree counting, node permutation); all float math runs on
    device.
"""

import numpy as np

from concourse import bacc, bass, mybir, tile
from concourse.bass_utils import run_bass_kernel_spmd

# ---------------------------------------------------------------- constants
P = 8                      # cores
N = 50000                  # nodes
IN_DIM = 128
HID = 64
OUT_DIM = 10
BLK = 128

F32 = mybir.dt.float32
I32 = mybir.dt.int32


def _derived():
    shard = N // P
    nblk = (shard + BLK - 1) // BLK
    pads = nblk * BLK
    tbl = P * pads
    return shard, nblk, pads, tbl


def _refresh_dims():
    global SHARD, NBLK, PADS, TBL
    SHARD, NBLK, PADS, TBL = _derived()


SHARD, NBLK, PADS, TBL = _derived()


# ------------------------------------------------------------- host prep
def _preprocess(x, edge_index):
    """Bucket edges into per-(core, dst-block) 128-edge chunks."""
    x = np.asarray(x, np.float32)
    ei = np.asarray(edge_index, np.int64)
    src, dst = ei[0], ei[1]

    deg = np.bincount(dst, minlength=N).astype(np.float32) + 1.0

    owner = dst // SHARD
    blk = (dst - owner * SHARD) // BLK
    dstl = ((dst - owner * SHARD) % BLK).astype(np.float32)
    s_own = src // SHARD
    s_loc = src % SHARD

    # chunk slots
    gid = owner * NBLK + blk
    order = np.argsort(gid, kind="stable")
    gid_s = gid[order]
    counts = np.bincount(gid_s, minlength=P * NBLK)
    starts = np.concatenate([[0], np.cumsum(counts)[:-1]])
    pos = np.arange(gid_s.size) - starts[gid_s]

    # per-block chunk count: max over cores (program must be core-uniform)
    C_arr = np.maximum(
        np.ceil(counts.reshape(P, NBLK).max(axis=0) / BLK).astype(np.int64), 1)
    base = np.concatenate([[0], np.cumsum(C_arr)[:-1]])
    T = int(C_arr.sum())

    own_s = gid_s // NBLK
    blk_s = gid_s % NBLK
    slot = base[blk_s] * BLK + pos            # slot within the core's stream

    # AG-table row (rank-ordered layout, layers 2/3)
    row23 = (s_own * PADS + s_loc)[order]
    dstl_s = dstl[order]

    g23 = np.zeros((P, T * BLK), np.int32)
    dv = np.full((P, T * BLK), -1.0, np.float32)
    flat = own_s * (T * BLK) + slot
    g23.reshape(-1)[flat] = row23.astype(np.int32)
    dv.reshape(-1)[flat] = dstl_s

    # layer-1 table row: per-core permuted layout, own shard first
    g1 = np.zeros((P, T * BLK), np.int32)
    s_own_s = s_own[order]
    s_loc_s = s_loc[order]
    for k in range(P):
        sel = own_s == k
        so = s_own_s[sel]
        # position of shard `so` in core k's permuted order [k, 0,1,..(!k)..,7]
        rank = np.where(so == k, 0, 1 + so - (so > k))
        g1.reshape(-1)[flat[sel]] = (rank * PADS + s_loc_s[sel]).astype(np.int32)

    x_pad = np.zeros((P, PADS, IN_DIM), np.float32)
    deg_pad = np.ones((P, PADS), np.float32)
    for k in range(P):
        x_pad[k, :SHARD] = x[k * SHARD:(k + 1) * SHARD]
        deg_pad[k, :SHARD] = deg[k * SHARD:(k + 1) * SHARD]

    per_core = []
    for k in range(P):
        perm = [k] + [c for c in range(P) if c != k]
        xp = x_pad[perm].reshape(TBL, IN_DIM)
        degp = deg_pad[perm].reshape(P * NBLK, BLK).T   # [128, 392]
        per_core.append(dict(
            xpt=np.ascontiguousarray(xp.T),
            degp=np.ascontiguousarray(degp),
            g1=np.ascontiguousarray(g1[k].reshape(T, BLK).T.astype(np.int32)),
            g23=np.ascontiguousarray(g23[k].reshape(T, BLK).T.astype(np.int32)),
            dstl=np.ascontiguousarray(dv[k].reshape(T, BLK).T),
        ))
    return per_core, tuple(int(c) for c in C_arr)


# ------------------------------------------------------------- device build
def _build(C_arr):
    T = int(sum(C_arr))
    c_base = [0]
    for c in C_arr[:-1]:
        c_base.append(c_base[-1] + c)
    NFULL = P * NBLK          # 392 blocks in the full node space

    nc = bacc.Bacc("TRN2", target_bir_lowering=False, debug=False,
                   enable_asserts=False, num_devices=P,
                   dynamic_dma_scratch_size=65536)

    xpt_d = nc.dram_tensor("xpt", [IN_DIM, TBL], F32, kind="ExternalInput").ap()
    degp_d = nc.dram_tensor("degp", [BLK, NFULL], F32, kind="ExternalInput").ap()
    g1_d = nc.dram_tensor("g1", [BLK, T], I32, kind="ExternalInput").ap()
    g23_d = nc.dram_tensor("g23", [BLK, T], I32, kind="ExternalInput").ap()
    dstl_d = nc.dram_tensor("dstl", [BLK, T], F32, kind="ExternalInput").ap()
    w_d = [nc.dram_tensor(f"w{i}", [d, HID if i < 3 else OUT_DIM], F32,
                          kind="ExternalInput").ap()
           for i, d in enumerate([IN_DIM, HID, HID, HID])]
    bt_d = [nc.dram_tensor(f"bt{i}", [BLK, HID if i < 3 else OUT_DIM], F32,
                           kind="ExternalInput").ap()
            for i in range(4)]
    iota_d = nc.dram_tensor("iota", [BLK, BLK], F32, kind="ExternalInput").ap()
    iden_d = nc.dram_tensor("iden", [BLK, BLK], F32, kind="ExternalInput").ap()
    out_d = nc.dram_tensor("probs", [PADS, OUT_DIM], F32, kind="ExternalOutput").ap()

    rg = [list(range(P))]

    with tile.TileContext(nc) as tc:
        with (
            tc.tile_pool(name="const", bufs=1) as cp,
            tc.tile_pool(name="xin", bufs=3) as xp_pool,
            tc.tile_pool(name="ht", bufs=3) as hp,
            tc.tile_pool(name="zt", bufs=3) as zp,
            tc.tile_pool(name="oh", bufs=12) as ohp,
            tc.tile_pool(name="msg", bufs=32) as mp,
            tc.tile_pool(name="fin", bufs=2) as fp,
            tc.tile_pool(name="pstp", bufs=2, space="PSUM") as pstp,
            tc.tile_pool(name="psacc", bufs=4, space="PSUM") as psacc,
            tc.tile_pool(name="dram", bufs=1, space="DRAM") as dp,
        ):
            # ---- constants into SBUF
            w_sb, bt_sb = [], []
            for i in range(4):
                wt = cp.tile(list(w_d[i].shape), F32, tag=f"w{i}", name=f"w{i}")
                nc.sync.dma_start(wt[:], w_d[i])
                w_sb.append(wt)
                bt = cp.tile(list(bt_d[i].shape), F32, tag=f"bt{i}", name=f"bt{i}")
                nc.sync.dma_start(bt[:], bt_d[i])
                bt_sb.append(bt)
            iota_sb = cp.tile([BLK, BLK], F32, tag="iota")
            nc.sync.dma_start(iota_sb[:], iota_d)
            iden_sb = cp.tile([BLK, BLK], F32, tag="iden")
            nc.sync.dma_start(iden_sb[:], iden_d)
            g1_sb = cp.tile([BLK, T], I32, tag="g1")
            nc.sync.dma_start(g1_sb[:], g1_d)
            g23_sb = cp.tile([BLK, T], I32, tag="g23")
            nc.sync.dma_start(g23_sb[:], g23_d)
            dstl_sb = cp.tile([BLK, T], F32, tag="dstl")
            nc.sync.dma_start(dstl_sb[:], dstl_d)

            deg_sb = cp.tile([BLK, NFULL], F32, tag="deg")
            nc.sync.dma_start(deg_sb[:], degp_d)
            dinv_sb = cp.tile([BLK, NFULL], F32, tag="dinv")
            nc.vector.reciprocal(dinv_sb[:], deg_sb[:])
            nc.scalar.activation(dinv_sb[:], dinv_sb[:],
                                 mybir.ActivationFunctionType.Sqrt)

            h_sb = [cp.tile([BLK, NBLK * HID], F32, tag=f"h{i}", name=f"h{i}")
                    for i in range(2)]
            zt_own = cp.tile([BLK, NBLK * HID], F32, tag="zt_own")

            def transform_block(src_ap, d_in, w_t, b, zdst):
                """z~_block = dinv[:,b] * (src_block @ W) -> zdst [128, HID]"""
                tp_ps = pstp.tile([d_in, BLK], F32, tag="tp", name="tp")
                nc.tensor.transpose(tp_ps[:], src_ap, iden_sb[:])
                hT = hp.tile([d_in, BLK], F32, tag="hT", name="hT")
                nc.vector.tensor_copy(hT[:], tp_ps[:])
                z_ps = psacc.tile([BLK, HID], F32, tag="acc", name="z_ps")
                nc.tensor.matmul(z_ps[:], hT[:], w_t[:], start=True, stop=True)
                nc.vector.tensor_scalar(zdst, z_ps[:], dinv_sb[:, b:b + 1],
                                        None, mybir.AluOpType.mult)

            def readout_block(h_ap, b):
                tp_ps = pstp.tile([HID, BLK], F32, tag="tp", name="tp")
                nc.tensor.transpose(tp_ps[:], h_ap, iden_sb[:])
                hT = hp.tile([HID, BLK], F32, tag="hT", name="hT")
                nc.vector.tensor_copy(hT[:], tp_ps[:])
                o_ps = psacc.tile([BLK, OUT_DIM], F32, tag="acc", name="o_ps")
                nc.tensor.matmul(o_ps[:], hT[:], w_sb[3][:],
                                 start=True, stop=True)
                logit = fp.tile([BLK, OUT_DIM], F32, tag="logit", name="logit")
                nc.vector.tensor_tensor(logit[:], o_ps[:], bt_sb[3][:],
                                        mybir.AluOpType.add)
                nmx = fp.tile([BLK, 1], F32, tag="nmx", name="nmx")
                nc.vector.reduce_max(nmx[:], logit[:],
                                     axis=mybir.AxisListType.X, negate=True)
                ex = fp.tile([BLK, OUT_DIM], F32, tag="ex", name="ex")
                ssum = fp.tile([BLK, 1], F32, tag="ssum", name="ssum")
                nc.scalar.activation(ex[:], logit[:],
                                     mybir.ActivationFunctionType.Exp,
                                     bias=nmx[:], accum_out=ssum[:])
                rs = fp.tile([BLK, 1], F32, tag="rs", name="rs")
                nc.vector.reciprocal(rs[:], ssum[:])
                prob = fp.tile([BLK, OUT_DIM], F32, tag="prob", name="prob")
                nc.vector.tensor_scalar(prob[:], ex[:], rs[:], None,
                                        mybir.AluOpType.mult)
                nc.sync.dma_start(out_d[b * BLK:(b + 1) * BLK, :], prob[:])

            def propagate(gidx_sb, table, h_nxt, b_t, readout=False):
                for b in range(NBLK):
                    C_b = C_arr[b]
                    agg_ps = psacc.tile([BLK, HID], F32, tag="acc", name="agg_ps")
                    for c in range(C_b):
                        t = c_base[b] + c
                        msg = mp.tile([BLK, HID], F32, tag="msg", name="msg")
                        nc.gpsimd.indirect_dma_start(
                            out=msg[:], out_offset=None, in_=table[:],
                            in_offset=bass.IndirectOffsetOnAxis(
                                ap=gidx_sb[:, t:t + 1], axis=0))
                        oh = ohp.tile([BLK, BLK], F32, tag="oh", name="oh")
                        nc.vector.tensor_scalar(
                            oh[:], iota_sb[:], dstl_sb[:, t:t + 1], None,
                            mybir.AluOpType.is_equal)
                        nc.tensor.matmul(agg_ps[:], oh[:], msg[:],
                                         start=(c == 0), stop=(c == C_b - 1))
                    sl = slice(b * HID, (b + 1) * HID)
                    tot = zp.tile([BLK, HID], F32, tag="tot", name="tot")
                    nc.vector.tensor_tensor(tot[:], agg_ps[:], zt_own[:, sl],
                                            mybir.AluOpType.add)
                    nc.vector.scalar_tensor_tensor(
                        h_nxt[:, sl], tot[:], dinv_sb[:, b:b + 1], b_t[:],
                        mybir.AluOpType.mult, mybir.AluOpType.add)
                    nc.scalar.activation(h_nxt[:, sl], h_nxt[:, sl],
                                         mybir.ActivationFunctionType.Relu)
                    if readout:
                        readout_block(h_nxt[:, sl], b)

            # ---------------- layer 1: full local table (x replicated)
            # batched 8-block staging keeps the sync engine off the critical
            # path (one 512KB load + one 256KB store per 8 blocks)
            table1 = dp.tile([TBL, HID], F32, tag="tbl0")
            GB = 8
            for g in range(NFULL // GB):
                # x arrives pre-transposed: columns are nodes, so each block
                # slice is directly the matmul's stationary operand
                xg = xp_pool.tile([IN_DIM, GB * BLK], F32, tag="xb", name="xb")
                nc.sync.dma_start(xg[:], xpt_d[:, g * GB * BLK:(g + 1) * GB * BLK])
                zg = zp.tile([BLK, GB * HID], F32, tag="zd", name="zd")
                for j in range(GB):
                    b = g * GB + j
                    z_ps = psacc.tile([BLK, HID], F32, tag="acc", name="z_ps")
                    nc.tensor.matmul(z_ps[:], xg[:, j * BLK:(j + 1) * BLK],
                                     w_sb[0][:], start=True, stop=True)
                    nc.vector.tensor_scalar(zg[:, j * HID:(j + 1) * HID],
                                            z_ps[:], dinv_sb[:, b:b + 1],
                                            None, mybir.AluOpType.mult)
                    if b < NBLK:
                        nc.vector.tensor_copy(
                            zt_own[:, b * HID:(b + 1) * HID],
                            zg[:, j * HID:(j + 1) * HID])
                nc.sync.dma_start(
                    table1[g * GB * BLK:(g + 1) * GB * BLK, :].rearrange(
                        "(j p) f -> p j f", p=BLK),
                    zg[:].rearrange("p (j f) -> p j f", f=HID))
            propagate(g1_sb, table1, h_sb[0], bt_sb[0])

            # ---------------- layers 2, 3: shard transform + AllGather
            for li in (1, 2):
                h_cur = h_sb[(li + 1) % 2]
                h_nxt = h_sb[li % 2]
                for b in range(NBLK):
                    transform_block(h_cur[:, b * HID:(b + 1) * HID], HID,
                                    w_sb[li], b,
                                    zt_own[:, b * HID:(b + 1) * HID])
                ag_in = dp.tile([PADS, HID], F32, tag=f"agin{li}",
                                name=f"agin{li}")
                nc.sync.dma_start(
                    ag_in[:].rearrange("(b p) f -> p b f", p=BLK),
                    zt_own[:].rearrange("p (b f) -> p b f", f=HID))
                table = dp.tile([TBL, HID], F32, tag=f"tbl{li}",
                                name=f"table{li}", addr_space="Shared")
                nc.gpsimd.collective_compute(
                    "AllGather", mybir.AluOpType.bypass, replica_groups=rg,
                    ins=[ag_in.opt()], outs=[table.opt()])
                propagate(g23_sb, table, h_nxt, bt_sb[li], readout=(li == 2))

    nc.compile()
    return nc


# ------------------------------------------------------------- entry point
_CACHE = {}


def _get_program(C_arr):
    if C_arr not in _CACHE:
        _CACHE[C_arr] = _build(C_arr)
    return _CACHE[C_arr]


def kernel(x, edge_index, W1, b1, W2, b2, W3, b3, Wr, br, trace=False):
    per_core, C_arr = _preprocess(x, edge_index)
    nc = _get_program(C_arr)

    ws = [np.asarray(w, np.float32) for w in (W1, W2, W3, Wr)]
    bts = [np.tile(np.asarray(b, np.float32).reshape(1, -1), (BLK, 1))
           for b in (b1, b2, b3, br)]
    iota = np.tile(np.arange(BLK, dtype=np.float32), (BLK, 1))
    iden = np.eye(BLK, dtype=np.float32)

    in_maps = []
    for k in range(P):
        m = dict(per_core[k])
        for i in range(4):
            m[f"w{i}"] = ws[i]
            m[f"bt{i}"] = bts[i]
        m["iota"] = iota
        m["iden"] = iden
        in_maps.append(m)

    res = run_bass_kernel_spmd(nc, in_maps, core_ids=list(range(P)),
                               trace=trace)
    out = np.empty((N, OUT_DIM), np.float32)
    for k in range(P):
        out[k * SHARD:(k + 1) * SHARD] = res.results[k]["probs"][:SHARD]
    kernel.last_results = res
    return out


# revision 19
# speedup vs baseline: 1.2269x; 1.0616x over previous
"""GCN (3-layer + readout) on 8 Trainium2 NeuronCores.

Strategy (dst-node sharding, 1D graph parallel):
  - Nodes are sharded across 8 cores (6250/core, padded to 6272 = 49 blocks
    of 128).  Each core aggregates messages for the edges whose dst lands in
    its shard.
  - Per layer: transform z = h @ W (PE), scale rows by dinv = deg^-1/2 so
    table rows are dinv[src]*z[src].  Layer 1's table is computed fully
    locally by every core (x is replicated; each core gets a node
    permutation with its own shard first so the self-term slice is uniform
    across the SPMD program).  Layers 2/3 AllGather the shard tables.
  - Edge gathers: one indirect DMA (standard InstDMACopy dynamic-offset
    SWDGE path, int32 row ids, one row per partition) per 128-edge chunk.
  - Scatter-add on the TensorEngine: per chunk a one-hot
    onehot[e, d] = (dst_local[e] == d) is built with one DVE
    tensor_scalar(is_equal) against an iota row; psum[128d, 64] +=
    onehot^T @ msg accumulates the dst-block aggregation.
  - Self loops (PyG GCNConv implicit): the own-shard table slice is kept in
    SBUF and added to the block aggregate before the dst-side dinv scale,
    yielding exactly dinv^2 * z.
  - Host-side preprocessing is strictly index/metadata work (edge bucketing,
    padding, degree counting, node permutation); all float math runs on
    device.
"""

import numpy as np

from concourse import bacc, bass, mybir, tile
from concourse.bass_utils import run_bass_kernel_spmd

# ---------------------------------------------------------------- constants
P = 8                      # cores
N = 50000                  # nodes
IN_DIM = 128
HID = 64
OUT_DIM = 10
BLK = 128

F32 = mybir.dt.float32
I32 = mybir.dt.int32


def _derived():
    shard = N // P
    nblk = (shard + BLK - 1) // BLK
    pads = nblk * BLK
    tbl = P * pads
    return shard, nblk, pads, tbl


def _refresh_dims():
    global SHARD, NBLK, PADS, TBL
    SHARD, NBLK, PADS, TBL = _derived()


SHARD, NBLK, PADS, TBL = _derived()


# ------------------------------------------------------------- host prep
def _preprocess(x, edge_index):
    """Bucket edges into per-(core, dst-block) 128-edge chunks.

    Nodes are bin-packed into the P*NBLK (core, block) bins by in-degree
    (capacity-constrained LPT) so every bin carries ~the same edge count —
    this minimizes the uniform per-block chunk counts, which set the Q7
    gather-instruction floor.
    """
    import heapq

    x = np.asarray(x, np.float32)
    ei = np.asarray(edge_index, np.int64)
    src, dst = ei[0], ei[1]

    degE = np.bincount(dst, minlength=N).astype(np.int64)
    deg = (degE + 1).astype(np.float32)

    NBINS = P * NBLK
    order_n = np.argsort(-degE, kind="stable")
    heap = [(0, b) for b in range(NBINS)]
    heapq.heapify(heap)
    fill = np.zeros(NBINS, np.int64)
    node_bin = np.empty(N, np.int64)
    node_slot = np.empty(N, np.int64)
    for n in order_n:
        while True:
            s, b = heapq.heappop(heap)
            if fill[b] < BLK:
                break
        node_bin[n] = b
        node_slot[n] = fill[b]
        fill[b] += 1
        heapq.heappush(heap, (s + int(degE[n]), b))

    newid = node_bin * BLK + node_slot          # padded global row of each node

    owner = node_bin[dst] // NBLK
    blk = node_bin[dst] % NBLK
    dstl = node_slot[dst].astype(np.float32)
    s_own = node_bin[src] // NBLK
    s_loc = (node_bin[src] % NBLK) * BLK + node_slot[src]   # padded local row

    # chunk slots
    gid = owner * NBLK + blk
    order = np.argsort(gid, kind="stable")
    gid_s = gid[order]
    counts = np.bincount(gid_s, minlength=P * NBLK)
    starts = np.concatenate([[0], np.cumsum(counts)[:-1]])
    pos = np.arange(gid_s.size) - starts[gid_s]

    # per-block chunk count: max over cores (program must be core-uniform)
    C_arr = np.maximum(
        np.ceil(counts.reshape(P, NBLK).max(axis=0) / BLK).astype(np.int64), 1)
    base = np.concatenate([[0], np.cumsum(C_arr)[:-1]])
    T = int(C_arr.sum())

    own_s = gid_s // NBLK
    blk_s = gid_s % NBLK
    slot = base[blk_s] * BLK + pos            # slot within the core's stream

    # AG-table row (rank-ordered layout, layers 2/3)
    row23 = (s_own * PADS + s_loc)[order]
    dstl_s = dstl[order]

    g23 = np.zeros((P, T * BLK), np.int32)
    dv = np.full((P, T * BLK), -1.0, np.float32)
    flat = own_s * (T * BLK) + slot
    g23.reshape(-1)[flat] = row23.astype(np.int32)
    dv.reshape(-1)[flat] = dstl_s

    # layer-1 table row: per-core permuted layout, own shard first
    g1 = np.zeros((P, T * BLK), np.int32)
    s_own_s = s_own[order]
    s_loc_s = s_loc[order]
    for k in range(P):
        sel = own_s == k
        so = s_own_s[sel]
        # position of shard `so` in core k's permuted order [k, 0,1,..(!k)..,7]
        rank = np.where(so == k, 0, 1 + so - (so > k))
        g1.reshape(-1)[flat[sel]] = (rank * PADS + s_loc_s[sel]).astype(np.int32)

    x_pad = np.zeros((P, PADS, IN_DIM), np.float32)
    deg_pad = np.ones((P, PADS), np.float32)
    x_pad[newid // PADS, newid % PADS] = x
    deg_pad[newid // PADS, newid % PADS] = deg

    per_core = []
    for k in range(P):
        perm = [k] + [c for c in range(P) if c != k]
        xp = x_pad[perm].reshape(TBL, IN_DIM)
        degp = deg_pad[perm].reshape(P * NBLK, BLK).T   # [128, 392]
        per_core.append(dict(
            xpt=np.ascontiguousarray(xp.T),
            degp=np.ascontiguousarray(degp),
            g1=np.ascontiguousarray(g1[k].reshape(T, BLK).T.astype(np.int32)),
            g23=np.ascontiguousarray(g23[k].reshape(T, BLK).T.astype(np.int32)),
            dstl=np.ascontiguousarray(dv[k].reshape(T, BLK).T),
        ))
    return per_core, tuple(int(c) for c in C_arr), newid


# ------------------------------------------------------------- device build
def _build(C_arr):
    T = int(sum(C_arr))
    c_base = [0]
    for c in C_arr[:-1]:
        c_base.append(c_base[-1] + c)
    NFULL = P * NBLK          # 392 blocks in the full node space

    nc = bacc.Bacc("TRN2", target_bir_lowering=False, debug=False,
                   enable_asserts=False, num_devices=P,
                   dynamic_dma_scratch_size=65536)

    xpt_d = nc.dram_tensor("xpt", [IN_DIM, TBL], F32, kind="ExternalInput").ap()
    degp_d = nc.dram_tensor("degp", [BLK, NFULL], F32, kind="ExternalInput").ap()
    g1_d = nc.dram_tensor("g1", [BLK, T], I32, kind="ExternalInput").ap()
    g23_d = nc.dram_tensor("g23", [BLK, T], I32, kind="ExternalInput").ap()
    dstl_d = nc.dram_tensor("dstl", [BLK, T], F32, kind="ExternalInput").ap()
    w_d = [nc.dram_tensor(f"w{i}", [d, HID if i < 3 else OUT_DIM], F32,
                          kind="ExternalInput").ap()
           for i, d in enumerate([IN_DIM, HID, HID, HID])]
    bt_d = [nc.dram_tensor(f"bt{i}", [BLK, HID if i < 3 else OUT_DIM], F32,
                           kind="ExternalInput").ap()
            for i in range(4)]
    iota_d = nc.dram_tensor("iota", [BLK, BLK], F32, kind="ExternalInput").ap()
    iden_d = nc.dram_tensor("iden", [BLK, BLK], F32, kind="ExternalInput").ap()
    out_d = nc.dram_tensor("probs", [PADS, OUT_DIM], F32, kind="ExternalOutput").ap()

    rg = [list(range(P))]

    with tile.TileContext(nc) as tc:
        with (
            tc.tile_pool(name="const", bufs=1) as cp,
            tc.tile_pool(name="xin", bufs=3) as xp_pool,
            tc.tile_pool(name="ht", bufs=3) as hp,
            tc.tile_pool(name="zt", bufs=3) as zp,
            tc.tile_pool(name="oh", bufs=12) as ohp,
            tc.tile_pool(name="msg", bufs=32) as mp,
            tc.tile_pool(name="fin", bufs=2) as fp,
            tc.tile_pool(name="pstp", bufs=2, space="PSUM") as pstp,
            tc.tile_pool(name="psacc", bufs=4, space="PSUM") as psacc,
            tc.tile_pool(name="dram", bufs=1, space="DRAM") as dp,
        ):
            # ---- constants into SBUF
            w_sb, bt_sb = [], []
            for i in range(4):
                wt = cp.tile(list(w_d[i].shape), F32, tag=f"w{i}", name=f"w{i}")
                nc.sync.dma_start(wt[:], w_d[i])
                w_sb.append(wt)
                bt = cp.tile(list(bt_d[i].shape), F32, tag=f"bt{i}", name=f"bt{i}")
                nc.sync.dma_start(bt[:], bt_d[i])
                bt_sb.append(bt)
            iota_sb = cp.tile([BLK, BLK], F32, tag="iota")
            nc.sync.dma_start(iota_sb[:], iota_d)
            iden_sb = cp.tile([BLK, BLK], F32, tag="iden")
            nc.sync.dma_start(iden_sb[:], iden_d)
            g1_sb = cp.tile([BLK, T], I32, tag="g1")
            nc.sync.dma_start(g1_sb[:], g1_d)
            g23_sb = cp.tile([BLK, T], I32, tag="g23")
            nc.sync.dma_start(g23_sb[:], g23_d)
            dstl_sb = cp.tile([BLK, T], F32, tag="dstl")
            nc.sync.dma_start(dstl_sb[:], dstl_d)

            deg_sb = cp.tile([BLK, NFULL], F32, tag="deg")
            nc.sync.dma_start(deg_sb[:], degp_d)
            dinv_sb = cp.tile([BLK, NFULL], F32, tag="dinv")
            nc.vector.reciprocal(dinv_sb[:], deg_sb[:])
            nc.scalar.activation(dinv_sb[:], dinv_sb[:],
                                 mybir.ActivationFunctionType.Sqrt)

            h_sb = [cp.tile([BLK, NBLK * HID], F32, tag=f"h{i}", name=f"h{i}")
                    for i in range(2)]
            zt_own = cp.tile([BLK, NBLK * HID], F32, tag="zt_own")

            def transform_block(src_ap, d_in, w_t, b, zdst):
                """z~_block = dinv[:,b] * (src_block @ W) -> zdst [128, HID]"""
                tp_ps = pstp.tile([d_in, BLK], F32, tag="tp", name="tp")
                nc.tensor.transpose(tp_ps[:], src_ap, iden_sb[:])
                hT = hp.tile([d_in, BLK], F32, tag="hT", name="hT")
                nc.vector.tensor_copy(hT[:], tp_ps[:])
                z_ps = psacc.tile([BLK, HID], F32, tag="acc", name="z_ps")
                nc.tensor.matmul(z_ps[:], hT[:], w_t[:], start=True, stop=True)
                nc.vector.tensor_scalar(zdst, z_ps[:], dinv_sb[:, b:b + 1],
                                        None, mybir.AluOpType.mult)

            def readout_block(h_ap, b):
                tp_ps = pstp.tile([HID, BLK], F32, tag="tp", name="tp")
                nc.tensor.transpose(tp_ps[:], h_ap, iden_sb[:])
                hT = hp.tile([HID, BLK], F32, tag="hT", name="hT")
                nc.vector.tensor_copy(hT[:], tp_ps[:])
                o_ps = psacc.tile([BLK, OUT_DIM], F32, tag="acc", name="o_ps")
                nc.tensor.matmul(o_ps[:], hT[:], w_sb[3][:],
                                 start=True, stop=True)
                logit = fp.tile([BLK, OUT_DIM], F32, tag="logit", name="logit")
                nc.vector.tensor_tensor(logit[:], o_ps[:], bt_sb[3][:],
                                        mybir.AluOpType.add)
                nmx = fp.tile([BLK, 1], F32, tag="nmx", name="nmx")
                nc.vector.reduce_max(nmx[:], logit[:],
                                     axis=mybir.AxisListType.X, negate=True)
                ex = fp.tile([BLK, OUT_DIM], F32, tag="ex", name="ex")
                ssum = fp.tile([BLK, 1], F32, tag="ssum", name="ssum")
                nc.scalar.activation(ex[:], logit[:],
                                     mybir.ActivationFunctionType.Exp,
                                     bias=nmx[:], accum_out=ssum[:])
                rs = fp.tile([BLK, 1], F32, tag="rs", name="rs")
                nc.vector.reciprocal(rs[:], ssum[:])
                prob = fp.tile([BLK, OUT_DIM], F32, tag="prob", name="prob")
                nc.vector.tensor_scalar(prob[:], ex[:], rs[:], None,
                                        mybir.AluOpType.mult)
                nc.sync.dma_start(out_d[b * BLK:(b + 1) * BLK, :], prob[:])

            def propagate(gidx_sb, table, h_nxt, b_t, readout=False):
                for b in range(NBLK):
                    C_b = C_arr[b]
                    agg_ps = psacc.tile([BLK, HID], F32, tag="acc", name="agg_ps")
                    for c in range(C_b):
                        t = c_base[b] + c
                        msg = mp.tile([BLK, HID], F32, tag="msg", name="msg")
                        nc.gpsimd.indirect_dma_start(
                            out=msg[:], out_offset=None, in_=table[:],
                            in_offset=bass.IndirectOffsetOnAxis(
                                ap=gidx_sb[:, t:t + 1], axis=0))
                        oh = ohp.tile([BLK, BLK], F32, tag="oh", name="oh")
                        nc.vector.tensor_scalar(
                            oh[:], iota_sb[:], dstl_sb[:, t:t + 1], None,
                            mybir.AluOpType.is_equal)
                        nc.tensor.matmul(agg_ps[:], oh[:], msg[:],
                                         start=(c == 0), stop=(c == C_b - 1))
                    sl = slice(b * HID, (b + 1) * HID)
                    tot = zp.tile([BLK, HID], F32, tag="tot", name="tot")
                    nc.vector.tensor_tensor(tot[:], agg_ps[:], zt_own[:, sl],
                                            mybir.AluOpType.add)
                    nc.vector.scalar_tensor_tensor(
                        h_nxt[:, sl], tot[:], dinv_sb[:, b:b + 1], b_t[:],
                        mybir.AluOpType.mult, mybir.AluOpType.add)
                    nc.scalar.activation(h_nxt[:, sl], h_nxt[:, sl],
                                         mybir.ActivationFunctionType.Relu)
                    if readout:
                        readout_block(h_nxt[:, sl], b)

            # ---------------- layer 1: full local table (x replicated)
            # batched 8-block staging keeps the sync engine off the critical
            # path (one 512KB load + one 256KB store per 8 blocks)
            table1 = dp.tile([TBL, HID], F32, tag="tbl0")
            GB = 8
            for g in range(NFULL // GB):
                # x arrives pre-transposed: columns are nodes, so each block
                # slice is directly the matmul's stationary operand
                xg = xp_pool.tile([IN_DIM, GB * BLK], F32, tag="xb", name="xb")
                nc.sync.dma_start(xg[:], xpt_d[:, g * GB * BLK:(g + 1) * GB * BLK])
                zg = zp.tile([BLK, GB * HID], F32, tag="zd", name="zd")
                for j in range(GB):
                    b = g * GB + j
                    z_ps = psacc.tile([BLK, HID], F32, tag="acc", name="z_ps")
                    nc.tensor.matmul(z_ps[:], xg[:, j * BLK:(j + 1) * BLK],
                                     w_sb[0][:], start=True, stop=True)
                    nc.vector.tensor_scalar(zg[:, j * HID:(j + 1) * HID],
                                            z_ps[:], dinv_sb[:, b:b + 1],
                                            None, mybir.AluOpType.mult)
                    if b < NBLK:
                        nc.vector.tensor_copy(
                            zt_own[:, b * HID:(b + 1) * HID],
                            zg[:, j * HID:(j + 1) * HID])
                nc.sync.dma_start(
                    table1[g * GB * BLK:(g + 1) * GB * BLK, :].rearrange(
                        "(j p) f -> p j f", p=BLK),
                    zg[:].rearrange("p (j f) -> p j f", f=HID))
            propagate(g1_sb, table1, h_sb[0], bt_sb[0])

            # ---------------- layers 2, 3: shard transform + AllGather
            for li in (1, 2):
                h_cur = h_sb[(li + 1) % 2]
                h_nxt = h_sb[li % 2]
                for b in range(NBLK):
                    transform_block(h_cur[:, b * HID:(b + 1) * HID], HID,
                                    w_sb[li], b,
                                    zt_own[:, b * HID:(b + 1) * HID])
                ag_in = dp.tile([PADS, HID], F32, tag=f"agin{li}",
                                name=f"agin{li}")
                nc.sync.dma_start(
                    ag_in[:].rearrange("(b p) f -> p b f", p=BLK),
                    zt_own[:].rearrange("p (b f) -> p b f", f=HID))
                table = dp.tile([TBL, HID], F32, tag=f"tbl{li}",
                                name=f"table{li}", addr_space="Shared")
                nc.gpsimd.collective_compute(
                    "AllGather", mybir.AluOpType.bypass, replica_groups=rg,
                    ins=[ag_in.opt()], outs=[table.opt()])
                propagate(g23_sb, table, h_nxt, bt_sb[li], readout=(li == 2))

    nc.compile()
    return nc


# ------------------------------------------------------------- entry point
_CACHE = {}


def _get_program(C_arr):
    if C_arr not in _CACHE:
        _CACHE[C_arr] = _build(C_arr)
    return _CACHE[C_arr]


def kernel(x, edge_index, W1, b1, W2, b2, W3, b3, Wr, br, trace=False):
    per_core, C_arr, newid = _preprocess(x, edge_index)
    nc = _get_program(C_arr)

    ws = [np.asarray(w, np.float32) for w in (W1, W2, W3, Wr)]
    bts = [np.tile(np.asarray(b, np.float32).reshape(1, -1), (BLK, 1))
           for b in (b1, b2, b3, br)]
    iota = np.tile(np.arange(BLK, dtype=np.float32), (BLK, 1))
    iden = np.eye(BLK, dtype=np.float32)

    in_maps = []
    for k in range(P):
        m = dict(per_core[k])
        for i in range(4):
            m[f"w{i}"] = ws[i]
            m[f"bt{i}"] = bts[i]
        m["iota"] = iota
        m["iden"] = iden
        in_maps.append(m)

    res = run_bass_kernel_spmd(nc, in_maps, core_ids=list(range(P)),
                               trace=trace)
    allp = np.concatenate([res.results[k]["probs"] for k in range(P)], axis=0)
    out = allp[newid]
    kernel.last_results = res
    return out
